# revision 50
# baseline (speedup 1.0000x reference)
"""Trainium2 Bass kernel for nn_Decoder: fused single-step LSTM decoder.

Reference computation (per token t of batch b, state never advances):
    gates = x[b,t] @ W_ih.T + (h0[b] @ W_hh.T + b_ih + b_hh)     # [2048]
    i, f, g, o = sigmoid/sigmoid/tanh/sigmoid of gate quarters
    c = f * c0[b] + i * g
    h = o * tanh(c)
    out[b,t] = h @ fc_w.T + fc_b                                 # [513]

Sharding: data-parallel, batch 64 -> 8 batches per core on 8 NeuronCores.

Per-core design ("mixed-precision gates: i/f/o fp8-hi DRSW, g bf16,
col512 as K=1 PE matmuls"):
  - One supertile = one batch = 1024 tokens. 8 supertiles per core.
  - Error budget analysis (validated by a numpy bit-sim that matches HW
    to 3 digits): the sigmoid gates i,f,o damp input noise ~4x
    (sigmoid' <= 0.25), the tanh gate g does not. So i/f/o use single
    fp8 e4m3 x and W with DoubleRowSwInterleave (2 k-tiles of 128
    feats per ~290ns pass; no residual correction), while g uses bf16
    x and W (4 plain K=128 passes, ~262ns each). Measured rel err
    1.55e-2 vs the baseline hi+lo-residual scheme's 1.71e-2, at 80
    gate passes/supertile instead of 128.
  - Feature 512 (D=513): the rank-1 term w512[gate]*x512[tok] is a
    K=1 bf16 matmul (~220ns) that closes each chunk's PSUM accum
    group. Ablation showed the DVE scalar_tensor_tensor alternative
    (psum operand -> 1x mode, ~1.2us/chunk) saturates DVE and
    serializes the PE->DVE->ACT chain; the K=1 PE tax (7us/st) is
    cheaper than the measured DVE leakage.
  - Engine-ladder measurements (min-slope of reps=129 vs 513):
    matmuls-only 268us, +activations 315us, +cell+fc 324us (fc fills
    PE idle; roofline), +out-DMA 387->340 after the two DMA fixes:
    out rows padded to 528 floats (2112B, 64B-aligned bursts) and
    out-DMA issue moved to the GpSimd software DGE queue so SP's
    hardware queues carry only the x prefetch stream.
  - 3 gate-PSUM bufs (psum3) + 1-bank fc PSUM tiles keep the PE 2+
    chunks ahead of the trailing ScalarE activations.
  - The timing rep-loop body holds 2 ring periods (unr2): For_i places
    an all-engine barrier per iteration, so doubling the body halves
    the per-iteration drain + HAM re-warm seam (~10us/iter measured).
  - The per-batch bias (h0 @ W_hh.T + b_ih + b_hh, fp32 on host) and
    the 1/64 descale (fp8 chunks only) ride the ScalarE activation.
  - LSTM cell feature-major: GPSIMD does t1 = i*g (bf16); DVE does the
    fused c = f*c0 + t1 (scalar_tensor_tensor) and h = o*tanh(c) in
    bf16; ScalarE does tanh(c). All hidden under the PE.
  - fc in bf16 (h bf16 is the lhsT): 4 K-chunk matmuls per (128-token,
    258-col half); no on-device bias - fc_b is added in fp32 on the
    host after the gather. DVE copies PSUM->SBUF; DMA out per subtile.
  - fc for supertile s is emitted interleaved into supertile s+1's
    gate chunk loop; matmuls are weight-major so both token halves of
    a weight slice are adjacent.
"""

from contextlib import ExitStack

import ml_dtypes
import numpy as np

import concourse.bass as bass
import concourse.tile as tile
from concourse import bacc, mybir
from concourse.bass_utils import run_bass_kernel_spmd

FP32 = mybir.dt.float32
BF16 = mybir.dt.bfloat16
FP8 = mybir.dt.float8e4
AFT = mybir.ActivationFunctionType
ALU = mybir.AluOpType
DRSW = mybir.MatmulPerfMode.DoubleRowSwInterleave

N_CORES = 8
B, T, D = 64, 1024, 513
H = 512
B_LOC = B // N_CORES  # 8 batches per core
TOK = B_LOC * T  # 8192 tokens per core
ST = T  # supertile = one batch = 1024 tokens
NST = TOK // ST  # 8 supertiles
DPAD = 516  # fc output padded width
NHALF = 258  # fc N split halves (each fits one PSUM bank)

SW = 64.0  # W_ih fp8 scale (PSUM = 64*gates)
E4 = ml_dtypes.float8_e4m3
BFD = ml_dtypes.bfloat16


DEFAULT_VARIANT = "colpe+psum3+odmag+outpad+unr2"


def build_nc(reps=1, mode="full", variant=None):
    if variant is None:
        variant = DEFAULT_VARIANT
    flags = set(variant.split("+")) if variant else set()
    nc = bacc.Bacc("TRN2", target_bir_lowering=False, debug=False, num_devices=N_CORES)
    xt8 = nc.dram_tensor("xt", [512, TOK], FP8, kind="ExternalInput").ap()
    xbf = nc.dram_tensor("xb", [512, TOK], BF16, kind="ExternalInput").ap()
    w8 = nc.dram_tensor("w8", [128, 2, 2048, 2], FP8, kind="ExternalInput").ap()
    wg = nc.dram_tensor("wg", [128, 4, 512], BF16, kind="ExternalInput").ap()
    any_pe_col = ("colpe" in flags) or ("colhalf" in flags)
    any_stt_col = ("colpe" not in flags) or ("colhalf" in flags)
    if any_pe_col:
        # rank-1 col512 on the PE: K=1 bf16 matmuls, row-vector operands
        x5h = nc.dram_tensor("x5h", [1, TOK], BF16, kind="ExternalInput").ap()
        wct = nc.dram_tensor("wct", [1, 2048], BF16, kind="ExternalInput").ap()
    if any_stt_col:
        x5 = nc.dram_tensor("x5", [1, TOK], FP32, kind="ExternalInput").ap()
        wcol = nc.dram_tensor("wcol", [128, 16], FP32, kind="ExternalInput").ap()
    fcw = nc.dram_tensor("fcw", [128, 4, DPAD], BF16, kind="ExternalInput").ap()
    bct = nc.dram_tensor("bconst", [128, 16 * B_LOC], FP32, kind="ExternalInput").ap()
    c0t = nc.dram_tensor("c0t", [128, 4 * B_LOC], FP32, kind="ExternalInput").ap()
    if "obf16" in flags:
        # bf16 out, rows padded to 544*2B = 1088B = 17 aligned 64B bursts;
        # host upconverts and adds fc_b in fp32
        odt, owid = BF16, 544
    elif "outpad" in flags:
        odt, owid = FP32, 528  # 2112B = 33 aligned 64B bursts
    else:
        odt, owid = FP32, D
    out = nc.dram_tensor("out", [TOK, owid], odt, kind="ExternalOutput").ap()

    with tile.TileContext(nc) as tc, ExitStack() as ctx:
        const = ctx.enter_context(tc.tile_pool(name="const", bufs=1))
        xp8 = ctx.enter_context(tc.tile_pool(name="xp8", bufs=6))
        xpb = ctx.enter_context(tc.tile_pool(name="xpb", bufs=3))
        xp5r = ctx.enter_context(tc.tile_pool(name="xp5r", bufs=3))
        xp5b = ctx.enter_context(tc.tile_pool(name="xp5b", bufs=3))
        sigp = ctx.enter_context(tc.tile_pool(name="sig", bufs=8))
        tmpp = ctx.enter_context(tc.tile_pool(name="tmp", bufs=3))
        hp = ctx.enter_context(tc.tile_pool(name="h", bufs=12))
        outp = ctx.enter_context(tc.tile_pool(name="osb", bufs=4))
        if "gphalf" in flags:
            n_gpp = 6  # [128,512] half-chunk tiles, 1 bank each
        else:
            n_gpp = 3 if "psum3" in flags else 2
        gpp = ctx.enter_context(tc.tile_pool(name="gp", bufs=n_gpp, space="PSUM"))
        fmp = ctx.enter_context(tc.tile_pool(name="fm", bufs=2, space="PSUM"))
        gsp = (
            ctx.enter_context(tc.tile_pool(name="gs", bufs=3))
            if "sbout" in flags
            else None
        )

        def emit_x(st):
            """DMA supertile st's x: fp8 DR-pair tiles (feats 0..511) for
            i/f/o, bf16 k-tile stack for g, and the fp32 x512 row which is
            partition-broadcast on GpSimd for the DVE rank-1 col512 add."""
            ts = st * ST
            do_dma = "nox" not in flags
            xh = []
            for p in range(2):
                t = xp8.tile([128, 2, ST], FP8, tag="xpair", name=f"xh{p}")
                for i in range(2):
                    r = p * 256 + i * 128
                    if do_dma:
                        nc.sync.dma_start(t[:, i, :], xt8[r : r + 128, ts : ts + ST])
                xh.append(t)
            xb = xpb.tile([128, 4, ST], BF16, tag="xb")
            for kt in range(4):
                if do_dma:
                    nc.sync.dma_start(
                        xb[:, kt, :], xbf[kt * 128 : (kt + 1) * 128, ts : ts + ST]
                    )
            x5r16 = x5b = None
            if any_pe_col:
                x5r16 = xp5r.tile([1, ST], BF16, tag="x5r16")
                if do_dma:
                    nc.sync.dma_start(x5r16[:], x5h[:, ts : ts + ST])
            if any_stt_col:
                x5r = xp5r.tile([1, ST], FP32, tag="x5r")
                if do_dma:
                    nc.sync.dma_start(x5r[:], x5[:, ts : ts + ST])
                x5b = xp5b.tile([128, ST], FP32, tag="x5b")
                nc.gpsimd.partition_broadcast(x5b[:], x5r[:])
            return (xh[0], xh[1], xb, x5r16, x5b)

        # ---- startup: first supertiles' x, then weights ----
        x_queue = [emit_x(0)]

        w8_sb = const.tile([128, 2, 2048, 2], FP8, tag="w8")
        nc.sync.dma_start(w8_sb[:], w8)
        wg_sb = const.tile([128, 4, 512], BF16, tag="wg")
        nc.sync.dma_start(wg_sb[:], wg)
        if any_pe_col:
            wct_sb = const.tile([1, 2048], BF16, tag="wct")
            nc.sync.dma_start(wct_sb[:], wct)
        if any_stt_col:
            wcol_sb = const.tile([128, 16], FP32, tag="wcol")
            nc.sync.dma_start(wcol_sb[:], wcol)
        bct_sb = const.tile([128, 16 * B_LOC], FP32, tag="bct")
        c0_sb = const.tile([128, 4 * B_LOC], FP32, tag="c0")
        nc.sync.dma_start(bct_sb[:], bct)
        nc.sync.dma_start(c0_sb[:], c0t)
        fcw_sb = const.tile([128, 4, DPAD], BF16, tag="fcw")
        nc.sync.dma_start(fcw_sb[:], fcw)

        x_queue.append(emit_x(1))
        x_queue.append(emit_x(2 % NST))

        gate_funcs = [AFT.Sigmoid, AFT.Sigmoid, AFT.Tanh, AFT.Sigmoid]

        def emit_k_group(k, b, xtiles):
            """Gates + cell for h-chunk k of batch b. Returns h tile."""
            xh0, xh1, xb, x5r16, x5b = xtiles
            gs = []
            for gi in range(4):
                c = gi * 4 + k
                if "colpe" in flags:
                    colpe = True
                elif "colhalf" in flags:
                    colpe = (c % 2) == 0
                else:
                    colpe = False
                cs = slice(c * 128, (c + 1) * 128)
                if "gphalf" in flags:
                    gph = [gpp.tile([128, 512], FP32, tag="gp", name=f"gph{_h}")
                           for _h in range(2)]
                    gpv = lambda hf: gph[hf][:, 0:512]
                else:
                    gp = gpp.tile([128, 1024], FP32, tag="gp")
                    gpv = lambda hf: gp[:, hf * 512 : (hf + 1) * 512]
                if gi == 2:
                    # g gate: bf16, 4 plain K=128 passes per token half;
                    # weight-major so both halves of a k-tile are adjacent
                    for kt in range(4):
                        w = wg_sb[:, kt, k * 128 : (k + 1) * 128]
                        for hf in range(2):
                            hs = slice(hf * 512, (hf + 1) * 512)
                            nc.tensor.matmul(gpv(hf), w, xb[:, kt, hs],
                                             start=(kt == 0),
                                             stop=(kt == 3 and not colpe))
                else:
                    # i/f/o: single-fp8 DRSW, 2 passes per token half
                    mmspec = [(w8_sb[:, 0, cs, :], xh0), (w8_sb[:, 1, cs, :], xh1)]
                    for wi, (w, xt) in enumerate(mmspec):
                        for hf in range(2):
                            hs = slice(hf * 512, (hf + 1) * 512)
                            nc.tensor.matmul(gpv(hf), w, xt[:, :, hs],
                                             start=(wi == 0),
                                             stop=(wi == 1 and not colpe),
                                             perf_mode=DRSW)
                if colpe:
                    # rank-1 col512: K=1 bf16 matmul closes the accum group
                    for hf in range(2):
                        hs = slice(hf * 512, (hf + 1) * 512)
                        nc.tensor.matmul(gpv(hf), wct_sb[:, cs],
                                         x5r16[:, hs], start=False, stop=True)
                if mode == "mmonly":
                    continue
                if "gphalf" in flags:
                    assert colpe, "gphalf requires colpe"
                    s = sigp.tile([128, 1024], BF16, tag="sig")
                    for hf in range(2):
                        nc.scalar.activation(
                            s[:, hf * 512 : (hf + 1) * 512], gpv(hf),
                            gate_funcs[gi],
                            bias=bct_sb[:, c * B_LOC + b : c * B_LOC + b + 1],
                            scale=(1.0 if gi == 2 else 1.0 / SW),
                        )
                    gs.append(s)
                    continue
                # rank-1 col512 term added before the nonlinearity
                act_in = gp
                if not colpe and "nocol" not in flags:
                    eng = nc.gpsimd if "sttgp" in flags else nc.vector
                    if "sbout" in flags:
                        gsb = gsp.tile([128, 1024], FP32, tag="gsb")
                        eng.scalar_tensor_tensor(
                            gsb[:], in0=x5b[:], scalar=wcol_sb[:, c : c + 1],
                            in1=gp[:], op0=ALU.mult, op1=ALU.add,
                        )
                        act_in = gsb
                    else:
                        eng.scalar_tensor_tensor(
                            gp[:], in0=x5b[:], scalar=wcol_sb[:, c : c + 1],
                            in1=gp[:], op0=ALU.mult, op1=ALU.add,
                        )
                s = sigp.tile([128, 1024], BF16, tag="sig")
                nc.scalar.activation(
                    s[:], act_in[:], gate_funcs[gi],
                    bias=bct_sb[:, c * B_LOC + b : c * B_LOC + b + 1],
                    scale=(1.0 if gi == 2 else 1.0 / SW),
                )
                gs.append(s)
            if mode in ("mmonly", "noact"):
                return None
            i_s, f_s, g_s, o_s = gs
            t1 = tmpp.tile([128, 1024], BF16, tag="t1")
            nc.gpsimd.tensor_mul(t1[:], i_s[:], g_s[:])
            cc = tmpp.tile([128, 1024], FP32, tag="cc")
            # scalar_tensor_tensor is illegal on Pool/GpSimd (walrus
            # NCC_IXCG966) - cc stays on DVE
            nc.vector.scalar_tensor_tensor(
                cc[:], in0=f_s[:],
                scalar=c0_sb[:, k * B_LOC + b : k * B_LOC + b + 1],
                in1=t1[:], op0=ALU.mult, op1=ALU.add,
            )
            th = tmpp.tile([128, 1024], BF16, tag="th")
            nc.scalar.activation(th[:], cc[:], AFT.Tanh)
            h = hp.tile([128, 1024], BF16, tag="h")
            nc.vector.tensor_mul(h[:], o_s[:], th[:])
            return h

        def emit_fc(st, hn, m):
            """fc for 128-token subtile m of supertile st (bf16, no bias)."""
            if mode != "full":
                return
            ts = st * ST
            msl = slice(m * 128, (m + 1) * 128)
            if "psum3" in flags and "osb1" in flags:
                # 1-bank fc psum tiles, both halves copied into one osb
                # tile so each m-subtile ships as a single out-DMA
                osb = outp.tile([128, 2, NHALF], FP32, tag="osb")
                for nh in range(2):
                    pf = fmp.tile([128, 512], FP32, tag="fm")
                    nsl = slice(nh * NHALF, (nh + 1) * NHALF)
                    for kd in range(4):
                        nc.tensor.matmul(pf[:, 0:NHALF], hn[kd][:, msl],
                                         fcw_sb[:, kd, nsl],
                                         start=(kd == 0), stop=(kd == 3))
                    nc.vector.tensor_copy(osb[:, nh, :], pf[:, 0:NHALF])
                ncol = 516 if "outpad" in flags else D
                deng = nc.gpsimd if "odmag" in flags else nc.sync
                if "noout" not in flags:
                    deng.dma_start(
                        out[ts + m * 128 : ts + (m + 1) * 128, 0:ncol],
                        osb[:].rearrange("p a z -> p (a z)")[:, 0:ncol],
                    )
                return
            if "psum3" in flags:
                # 1-bank fc psum tiles (frees 2 banks for a 3rd gate buf)
                for nh in range(2):
                    pf = fmp.tile([128, 512], FP32, tag="fm")
                    nsl = slice(nh * NHALF, (nh + 1) * NHALF)
                    for kd in range(4):
                        nc.tensor.matmul(pf[:, 0:NHALF], hn[kd][:, msl],
                                         fcw_sb[:, kd, nsl],
                                         start=(kd == 0), stop=(kd == 3))
                    pad = "outpad" in flags or "obf16" in flags
                    ncol = NHALF if pad else (
                        NHALF if nh == 0 else D - NHALF)
                    osb = outp.tile([128, NHALF], odt, tag="osb")
                    if "rebal" in flags or "rebal3" in flags:
                        nc.scalar.copy(osb[:], pf[:, 0:NHALF])
                    elif "rebal2" in flags and (m + nh) % 2:
                        nc.scalar.copy(osb[:], pf[:, 0:NHALF])
                    else:
                        nc.vector.tensor_copy(osb[:], pf[:, 0:NHALF])
                    if "noout" not in flags:
                        deng = nc.sync
                        if "odmas" in flags:
                            deng = nc.scalar
                        elif "odmago" in flags:
                            deng = nc.gpsimd if (m + nh) % 2 else nc.sync
                        elif "odmag" in flags:
                            deng = nc.gpsimd
                        deng.dma_start(
                            out[ts + m * 128 : ts + (m + 1) * 128,
                                nh * NHALF : nh * NHALF + ncol],
                            osb[:, 0:ncol],
                        )
                return
            pf = fmp.tile([128, 1024], FP32, tag="fm")
            for kd in range(4):
                for nh in range(2):
                    po = pf[:, nh * 512 : nh * 512 + NHALF]
                    nsl = slice(nh * NHALF, (nh + 1) * NHALF)
                    nc.tensor.matmul(po, hn[kd][:, msl], fcw_sb[:, kd, nsl],
                                     start=(kd == 0), stop=(kd == 3))
            osb = outp.tile([128, 2, NHALF], odt, tag="osb")
            pfv = pf[:].rearrange("p (a z) -> p a z", a=2)[:, :, 0:NHALF]
            if "rebal" in flags:
                nc.scalar.copy(osb[:], pfv)
            else:
                nc.vector.tensor_copy(osb[:], pfv)
            if "noout" not in flags:
                deng = nc.sync
                if "odmas" in flags:
                    deng = nc.scalar
                elif "odmag" in flags:
                    deng = nc.gpsimd
                deng.dma_start(
                    out[ts + m * 128 : ts + (m + 1) * 128, 0:D],
                    osb[:].rearrange("p a z -> p (a z)")[:, 0:D],
                )

        # ---- prologue: gates+cell for supertile 0 (no fc yet) ----
        xtiles = x_queue.pop(0)
        h_prev = [emit_k_group(k, 0, xtiles) for k in range(4)]

        # ---- main loop: fc(st) interleaved with gates+cell(st+1) ----
        # unr2: 2 ring periods per For_i iteration halves the per-iteration
        # all-engine-barrier seam (and its HAM re-warm cost)
        unroll = 2 if ("unr2" in flags and reps > 1) else 1
        rep_ctx = (
            tc.For_i(0, reps // unroll, 1, staggered_reset=True)
            if reps > 1
            else None
        )
        if rep_ctx is not None:
            rep_ctx.__enter__()
        for _u in range(unroll):
            for st in range(NST):
                s_next = (st + 1) % NST
                do_gates = (reps > 1) or (st < NST - 1)
                if do_gates:
                    xtiles = x_queue.pop(0)
                    if reps > 1:
                        x_queue.append(emit_x((s_next + 2) % NST))
                    elif s_next + 2 < NST:
                        x_queue.append(emit_x(s_next + 2))
                h_new = []
                for k in range(4):
                    if do_gates:
                        h_new.append(emit_k_group(k, s_next, xtiles))
                    emit_fc(st, h_prev, 2 * k)
                    emit_fc(st, h_prev, 2 * k + 1)
                if do_gates:
                    h_prev = h_new
        if rep_ctx is not None:
            rep_ctx.__exit__(None, None, None)

    nc.compile()
    return nc


_NC_CACHE = []


def get_nc():
    if not _NC_CACHE:
        _NC_CACHE.append(build_nc())
    return _NC_CACHE[0]


def make_in_maps(decoder_inputs, h0, c0, W_ih, W_hh, b_ih, b_hh, fc_w, fc_b):
    di = np.asarray(decoder_inputs, dtype=np.float32)
    h0 = np.asarray(h0, dtype=np.float32)[0]  # [64, 512]
    c0 = np.asarray(c0, dtype=np.float32)[0]
    W_ih = np.asarray(W_ih, dtype=np.float32)
    W_hh = np.asarray(W_hh, dtype=np.float32)
    b_ih = np.asarray(b_ih, dtype=np.float32)
    b_hh = np.asarray(b_hh, dtype=np.float32)
    fc_w = np.asarray(fc_w, dtype=np.float32)

    bc = h0 @ W_hh.T + b_ih + b_hh  # [64, 2048]

    # i/f/o gate weights: single fp8 e4m3 at x64 scale over feats 0..511,
    # SwInterleave DR-pair layout. (g-gate slots are packed too but unused.)
    W_hi8 = (SW * W_ih[:, 0:512]).astype(E4)  # [2048, 512]

    def wpack(w8):
        # SwInterleave layout: arr[k, pair, c*128+j, i] = w8[c*128+(127-j),
        # pair*256 + i*128 + k]  -> [128, 2(pair), 2048(g), 2(ktile)]
        tmp = w8.reshape(16, 128, 2, 2, 128)  # (c, m, pair, i, k)
        return np.ascontiguousarray(
            np.flip(tmp, axis=1).transpose(4, 2, 0, 1, 3).reshape(128, 2, 2048, 2))

    w8_a = wpack(W_hi8)

    # g gate weights bf16: [128(k within ktile), 4(ktile), 512(g rows)]
    wg_a = np.ascontiguousarray(
        W_ih[1024:1536, 0:512].T.astype(BFD).reshape(4, 128, 512).transpose(1, 0, 2))

    # col512 scalars per chunk, pre-scaled by 64 for the fp8 (i/f/o) chunks
    wcol_a = np.empty((128, 16), dtype=np.float32)
    for c in range(16):
        s = SW if (c // 4) != 2 else 1.0
        wcol_a[:, c] = s * W_ih[c * 128 : (c + 1) * 128, 512]
    # row-vector form for the K=1 PE path (bf16)
    wct_a = np.ascontiguousarray(
        wcol_a.T.reshape(1, 2048).astype(BFD))

    # fc weights bf16, [128, 4(k-chunk), 516]
    fc_pad = np.zeros((512, DPAD), dtype=BFD)
    fc_pad[:, 0:D] = fc_w.T.astype(BFD)
    fcw_a = np.ascontiguousarray(
        fc_pad.reshape(4, 128, DPAD).transpose(1, 0, 2))

    in_maps = []
    for core in range(N_CORES):
        bs = core * B_LOC
        xc = di[bs : bs + B_LOC].reshape(TOK, D)
        xt8_a = np.ascontiguousarray(xc.T[0:512].astype(E4))  # [512, TOK]
        xbf_a = np.ascontiguousarray(xc.T[0:512].astype(BFD))  # [512, TOK]
        x5_a = np.ascontiguousarray(xc.T[512:513].astype(np.float32))  # [1, TOK]
        bct = np.ascontiguousarray(
            bc[bs : bs + B_LOC]
            .reshape(B_LOC, 16, 128)
            .transpose(2, 1, 0)
            .reshape(128, -1)
        )
        c0c = np.ascontiguousarray(
            c0[bs : bs + B_LOC]
            .reshape(B_LOC, 4, 128)
            .transpose(2, 1, 0)
            .reshape(128, -1)
        )
        in_maps.append(
            {
                "xt": xt8_a,
                "xb": xbf_a,
                "x5": x5_a,
                "x5h": np.ascontiguousarray(x5_a.astype(BFD)),
                "w8": w8_a,
                "wg": wg_a,
                "wcol": wcol_a,
                "wct": wct_a,
                "fcw": fcw_a,
                "bconst": bct,
                "c0t": c0c,
            }
        )
    return in_maps


def kernel(**inputs):
    in_maps = make_in_maps(**inputs)
    nc = get_nc()
    res = run_bass_kernel_spmd(nc, in_maps, core_ids=list(range(N_CORES)))
    out = np.concatenate(
        [res.results[c]["out"][:, 0:D].astype(np.float32) for c in range(N_CORES)],
        axis=0,
    )
    out = out.reshape(B, T, D)
    out += np.asarray(inputs["fc_b"], dtype=np.float32)  # exact fp32 bias
    return out


# revision 51
# speedup vs baseline: 1.0321x; 1.0321x over previous
"""Trainium2 Bass kernel for nn_Decoder: fused single-step LSTM decoder.

Reference computation (per token t of batch b, state never advances):
    gates = x[b,t] @ W_ih.T + (h0[b] @ W_hh.T + b_ih + b_hh)     # [2048]
    i, f, g, o = sigmoid/sigmoid/tanh/sigmoid of gate quarters
    c = f * c0[b] + i * g
    h = o * tanh(c)
    out[b,t] = h @ fc_w.T + fc_b                                 # [513]

Sharding: data-parallel, batch 64 -> 8 batches per core on 8 NeuronCores.

Per-core design ("mixed-precision gates: i/f/o fp8-hi DRSW, g bf16,
col512 as K=1 PE matmuls"):
  - One supertile = one batch = 1024 tokens. 8 supertiles per core.
  - Error budget analysis (validated by a numpy bit-sim that matches HW
    to 3 digits): the sigmoid gates i,f,o damp input noise ~4x
    (sigmoid' <= 0.25), the tanh gate g does not. So i/f/o use single
    fp8 e4m3 x and W with DoubleRowSwInterleave (2 k-tiles of 128
    feats per ~290ns pass; no residual correction), while g uses bf16
    x and W (4 plain K=128 passes, ~262ns each). Measured rel err
    1.55e-2 vs the baseline hi+lo-residual scheme's 1.71e-2, at 80
    gate passes/supertile instead of 128.
  - Feature 512 (D=513): the rank-1 term w512[gate]*x512[tok] is a
    K=1 bf16 matmul (~220ns) that closes each chunk's PSUM accum
    group. Ablation showed the DVE scalar_tensor_tensor alternative
    (psum operand -> 1x mode, ~1.2us/chunk) saturates DVE and
    serializes the PE->DVE->ACT chain; the K=1 PE tax (7us/st) is
    cheaper than the measured DVE leakage.
  - Engine-ladder measurements (min-slope of reps=129 vs 513):
    matmuls-only 268us, +activations 315us, +cell+fc 324us (fc fills
    PE idle; roofline), +out-DMA 387->340 after the two DMA fixes:
    out rows padded to 528 floats (2112B, 64B-aligned bursts) and
    out-DMA issue moved to the GpSimd software DGE queue so SP's
    hardware queues carry only the x prefetch stream.
  - 3 gate-PSUM bufs (psum3) + 1-bank fc PSUM tiles keep the PE 2+
    chunks ahead of the trailing ScalarE activations.
  - The timing rep-loop body holds 2 ring periods (unr2): For_i places
    an all-engine barrier per iteration, so doubling the body halves
    the per-iteration drain + HAM re-warm seam (~10us/iter measured).
  - The per-batch bias (h0 @ W_hh.T + b_ih + b_hh, fp32 on host) and
    the 1/64 descale (fp8 chunks only) ride the ScalarE activation.
  - LSTM cell feature-major: GPSIMD does t1 = i*g (bf16); DVE does the
    fused c = f*c0 + t1 (scalar_tensor_tensor) and h = o*tanh(c) in
    bf16; ScalarE does tanh(c). All hidden under the PE.
  - fc in bf16 (h bf16 is the lhsT): 4 K-chunk matmuls per (128-token,
    258-col half); no on-device bias - fc_b is added in fp32 on the
    host after the gather. DVE copies PSUM->SBUF; DMA out per subtile.
  - fc for supertile s is emitted interleaved into supertile s+1's
    gate chunk loop; matmuls are weight-major so both token halves of
    a weight slice are adjacent.
"""

from contextlib import ExitStack

import ml_dtypes
import numpy as np

import concourse.bass as bass
import concourse.tile as tile
from concourse import bacc, mybir
from concourse.bass_utils import run_bass_kernel_spmd

FP32 = mybir.dt.float32
BF16 = mybir.dt.bfloat16
FP8 = mybir.dt.float8e4
AFT = mybir.ActivationFunctionType
ALU = mybir.AluOpType
DRSW = mybir.MatmulPerfMode.DoubleRowSwInterleave

N_CORES = 8
B, T, D = 64, 1024, 513
H = 512
B_LOC = B // N_CORES  # 8 batches per core
TOK = B_LOC * T  # 8192 tokens per core
ST = T  # supertile = one batch = 1024 tokens
NST = TOK // ST  # 8 supertiles
DPAD = 516  # fc output padded width
NHALF = 258  # fc N split halves (each fits one PSUM bank)

SW = 64.0  # W_ih fp8 scale (PSUM = 64*gates)
E4 = ml_dtypes.float8_e4m3
BFD = ml_dtypes.bfloat16


DEFAULT_VARIANT = "colpe+psum3+odmag+outpad+unr2"


def build_nc(reps=1, mode="full", variant=None):
    if variant is None:
        variant = DEFAULT_VARIANT
    flags = set(variant.split("+")) if variant else set()
    nc = bacc.Bacc("TRN2", target_bir_lowering=False, debug=False, num_devices=N_CORES)
    xt8 = nc.dram_tensor("xt", [512, TOK], FP8, kind="ExternalInput").ap()
    xbf = nc.dram_tensor("xb", [512, TOK], BF16, kind="ExternalInput").ap()
    w8 = nc.dram_tensor("w8", [128, 2, 2048, 2], FP8, kind="ExternalInput").ap()
    wg = nc.dram_tensor("wg", [128, 4, 512], BF16, kind="ExternalInput").ap()
    any_pe_col = ("colpe" in flags) or ("colhalf" in flags)
    any_stt_col = ("colpe" not in flags) or ("colhalf" in flags)
    if any_pe_col:
        # rank-1 col512 on the PE: K=1 bf16 matmuls, row-vector operands
        x5h = nc.dram_tensor("x5h", [1, TOK], BF16, kind="ExternalInput").ap()
        wct = nc.dram_tensor("wct", [1, 2048], BF16, kind="ExternalInput").ap()
    if any_stt_col:
        x5 = nc.dram_tensor("x5", [1, TOK], FP32, kind="ExternalInput").ap()
        wcol = nc.dram_tensor("wcol", [128, 16], FP32, kind="ExternalInput").ap()
    fcw = nc.dram_tensor("fcw", [128, 4, DPAD], BF16, kind="ExternalInput").ap()
    bct = nc.dram_tensor("bconst", [128, 16 * B_LOC], FP32, kind="ExternalInput").ap()
    c0t = nc.dram_tensor("c0t", [128, 4 * B_LOC], FP32, kind="ExternalInput").ap()
    if "obf16" in flags:
        # bf16 out, rows padded to 544*2B = 1088B = 17 aligned 64B bursts;
        # host upconverts and adds fc_b in fp32
        odt, owid = BF16, 544
    elif "outpad" in flags:
        odt, owid = FP32, 528  # 2112B = 33 aligned 64B bursts
    else:
        odt, owid = FP32, D
    out = nc.dram_tensor("out", [TOK, owid], odt, kind="ExternalOutput").ap()

    with tile.TileContext(nc) as tc, ExitStack() as ctx:
        const = ctx.enter_context(tc.tile_pool(name="const", bufs=1))
        xp8 = ctx.enter_context(tc.tile_pool(name="xp8", bufs=6))
        xpb = ctx.enter_context(tc.tile_pool(name="xpb", bufs=3))
        xp5r = ctx.enter_context(tc.tile_pool(name="xp5r", bufs=3))
        xp5b = ctx.enter_context(tc.tile_pool(name="xp5b", bufs=3))
        big = "bufs2" in flags
        sigp = ctx.enter_context(tc.tile_pool(name="sig", bufs=16 if big else 8))
        tmpp = ctx.enter_context(tc.tile_pool(name="tmp", bufs=6 if big else 3))
        hp = ctx.enter_context(tc.tile_pool(name="h", bufs=12))
        outp = ctx.enter_context(tc.tile_pool(name="osb", bufs=6 if big else 4))
        if "gphalf" in flags:
            n_gpp = 6  # [128,512] half-chunk tiles, 1 bank each
        else:
            n_gpp = 3 if "psum3" in flags else 2
        gpp = ctx.enter_context(tc.tile_pool(name="gp", bufs=n_gpp, space="PSUM"))
        fmp = ctx.enter_context(tc.tile_pool(name="fm", bufs=2, space="PSUM"))
        gsp = (
            ctx.enter_context(tc.tile_pool(name="gs", bufs=3))
            if "sbout" in flags
            else None
        )

        def emit_x(st):
            """DMA supertile st's x: fp8 DR-pair tiles (feats 0..511) for
            i/f/o, bf16 k-tile stack for g, and the fp32 x512 row which is
            partition-broadcast on GpSimd for the DVE rank-1 col512 add."""
            ts = st * ST
            do_dma = "nox" not in flags
            xh = []
            for p in range(2):
                t = xp8.tile([128, 2, ST], FP8, tag="xpair", name=f"xh{p}")
                for i in range(2):
                    r = p * 256 + i * 128
                    if do_dma:
                        nc.sync.dma_start(t[:, i, :], xt8[r : r + 128, ts : ts + ST])
                xh.append(t)
            xb = xpb.tile([128, 4, ST], BF16, tag="xb")
            for kt in range(4):
                if do_dma:
                    nc.sync.dma_start(
                        xb[:, kt, :], xbf[kt * 128 : (kt + 1) * 128, ts : ts + ST]
                    )
            x5r16 = x5b = None
            if any_pe_col:
                x5r16 = xp5r.tile([1, ST], BF16, tag="x5r16")
                if do_dma:
                    nc.sync.dma_start(x5r16[:], x5h[:, ts : ts + ST])
            if any_stt_col:
                x5r = xp5r.tile([1, ST], FP32, tag="x5r")
                if do_dma:
                    nc.sync.dma_start(x5r[:], x5[:, ts : ts + ST])
                x5b = xp5b.tile([128, ST], FP32, tag="x5b")
                nc.gpsimd.partition_broadcast(x5b[:], x5r[:])
            return (xh[0], xh[1], xb, x5r16, x5b)

        # ---- startup: first supertiles' x, then weights ----
        x_queue = [emit_x(0)]

        w8_sb = const.tile([128, 2, 2048, 2], FP8, tag="w8")
        nc.sync.dma_start(w8_sb[:], w8)
        wg_sb = const.tile([128, 4, 512], BF16, tag="wg")
        nc.sync.dma_start(wg_sb[:], wg)
        if any_pe_col:
            wct_sb = const.tile([1, 2048], BF16, tag="wct")
            nc.sync.dma_start(wct_sb[:], wct)
        if any_stt_col:
            wcol_sb = const.tile([128, 16], FP32, tag="wcol")
            nc.sync.dma_start(wcol_sb[:], wcol)
        bct_sb = const.tile([128, 16 * B_LOC], FP32, tag="bct")
        c0_sb = const.tile([128, 4 * B_LOC], FP32, tag="c0")
        nc.sync.dma_start(bct_sb[:], bct)
        nc.sync.dma_start(c0_sb[:], c0t)
        fcw_sb = const.tile([128, 4, DPAD], BF16, tag="fcw")
        nc.sync.dma_start(fcw_sb[:], fcw)

        x_queue.append(emit_x(1))
        x_queue.append(emit_x(2 % NST))

        gate_funcs = [AFT.Sigmoid, AFT.Sigmoid, AFT.Tanh, AFT.Sigmoid]

        def emit_k_group(k, b, xtiles):
            """Gates + cell for h-chunk k of batch b. Returns h tile."""
            xh0, xh1, xb, x5r16, x5b = xtiles
            gs = []
            for gi in range(4):
                c = gi * 4 + k
                if "colpe" in flags:
                    colpe = True
                elif "colhalf" in flags:
                    colpe = (c % 2) == 0
                else:
                    colpe = False
                cs = slice(c * 128, (c + 1) * 128)
                if "gphalf" in flags:
                    gph = [gpp.tile([128, 512], FP32, tag="gp", name=f"gph{_h}")
                           for _h in range(2)]
                    gpv = lambda hf: gph[hf][:, 0:512]
                else:
                    gp = gpp.tile([128, 1024], FP32, tag="gp")
                    gpv = lambda hf: gp[:, hf * 512 : (hf + 1) * 512]
                if gi == 2:
                    # g gate: bf16, 4 plain K=128 passes per token half;
                    # weight-major so both halves of a k-tile are adjacent
                    for kt in range(4):
                        w = wg_sb[:, kt, k * 128 : (k + 1) * 128]
                        for hf in range(2):
                            hs = slice(hf * 512, (hf + 1) * 512)
                            nc.tensor.matmul(gpv(hf), w, xb[:, kt, hs],
                                             start=(kt == 0),
                                             stop=(kt == 3 and not colpe))
                else:
                    # i/f/o: single-fp8 DRSW, 2 passes per token half
                    mmspec = [(w8_sb[:, 0, cs, :], xh0), (w8_sb[:, 1, cs, :], xh1)]
                    for wi, (w, xt) in enumerate(mmspec):
                        for hf in range(2):
                            hs = slice(hf * 512, (hf + 1) * 512)
                            nc.tensor.matmul(gpv(hf), w, xt[:, :, hs],
                                             start=(wi == 0),
                                             stop=(wi == 1 and not colpe),
                                             perf_mode=DRSW)
                if colpe:
                    # rank-1 col512: K=1 bf16 matmul closes the accum group
                    for hf in range(2):
                        hs = slice(hf * 512, (hf + 1) * 512)
                        nc.tensor.matmul(gpv(hf), wct_sb[:, cs],
                                         x5r16[:, hs], start=False, stop=True)
                if mode == "mmonly":
                    continue
                if "gphalf" in flags:
                    assert colpe, "gphalf requires colpe"
                    s = sigp.tile([128, 1024], BF16, tag="sig")
                    for hf in range(2):
                        nc.scalar.activation(
                            s[:, hf * 512 : (hf + 1) * 512], gpv(hf),
                            gate_funcs[gi],
                            bias=bct_sb[:, c * B_LOC + b : c * B_LOC + b + 1],
                            scale=(1.0 if gi == 2 else 1.0 / SW),
                        )
                    gs.append(s)
                    continue
                # rank-1 col512 term added before the nonlinearity
                act_in = gp
                if not colpe and "nocol" not in flags:
                    eng = nc.gpsimd if "sttgp" in flags else nc.vector
                    if "sbout" in flags:
                        gsb = gsp.tile([128, 1024], FP32, tag="gsb")
                        eng.scalar_tensor_tensor(
                            gsb[:], in0=x5b[:], scalar=wcol_sb[:, c : c + 1],
                            in1=gp[:], op0=ALU.mult, op1=ALU.add,
                        )
                        act_in = gsb
                    else:
                        eng.scalar_tensor_tensor(
                            gp[:], in0=x5b[:], scalar=wcol_sb[:, c : c + 1],
                            in1=gp[:], op0=ALU.mult, op1=ALU.add,
                        )
                s = sigp.tile([128, 1024], BF16, tag="sig")
                nc.scalar.activation(
                    s[:], act_in[:], gate_funcs[gi],
                    bias=bct_sb[:, c * B_LOC + b : c * B_LOC + b + 1],
                    scale=(1.0 if gi == 2 else 1.0 / SW),
                )
                gs.append(s)
            if mode in ("mmonly", "noact"):
                return None
            i_s, f_s, g_s, o_s = gs
            t1 = tmpp.tile([128, 1024], BF16, tag="t1")
            nc.gpsimd.tensor_mul(t1[:], i_s[:], g_s[:])
            cc = tmpp.tile([128, 1024], FP32, tag="cc")
            # scalar_tensor_tensor is illegal on Pool/GpSimd (walrus
            # NCC_IXCG966) - cc stays on DVE
            nc.vector.scalar_tensor_tensor(
                cc[:], in0=f_s[:],
                scalar=c0_sb[:, k * B_LOC + b : k * B_LOC + b + 1],
                in1=t1[:], op0=ALU.mult, op1=ALU.add,
            )
            th = tmpp.tile([128, 1024], BF16, tag="th")
            nc.scalar.activation(th[:], cc[:], AFT.Tanh)
            h = hp.tile([128, 1024], BF16, tag="h")
            nc.vector.tensor_mul(h[:], o_s[:], th[:])
            return h

        def emit_fc(st, hn, m):
            """fc for 128-token subtile m of supertile st (bf16, no bias)."""
            if mode != "full":
                return
            ts = st * ST
            msl = slice(m * 128, (m + 1) * 128)
            if "psum3" in flags and "osb1" in flags:
                # 1-bank fc psum tiles, both halves copied into one osb
                # tile so each m-subtile ships as a single out-DMA
                osb = outp.tile([128, 2, NHALF], FP32, tag="osb")
                for nh in range(2):
                    pf = fmp.tile([128, 512], FP32, tag="fm")
                    nsl = slice(nh * NHALF, (nh + 1) * NHALF)
                    for kd in range(4):
                        nc.tensor.matmul(pf[:, 0:NHALF], hn[kd][:, msl],
                                         fcw_sb[:, kd, nsl],
                                         start=(kd == 0), stop=(kd == 3))
                    nc.vector.tensor_copy(osb[:, nh, :], pf[:, 0:NHALF])
                ncol = 516 if "outpad" in flags else D
                deng = nc.gpsimd if "odmag" in flags else nc.sync
                if "noout" not in flags:
                    deng.dma_start(
                        out[ts + m * 128 : ts + (m + 1) * 128, 0:ncol],
                        osb[:].rearrange("p a z -> p (a z)")[:, 0:ncol],
                    )
                return
            if "psum3" in flags:
                # 1-bank fc psum tiles (frees 2 banks for a 3rd gate buf)
                for nh in range(2):
                    pf = fmp.tile([128, 512], FP32, tag="fm")
                    nsl = slice(nh * NHALF, (nh + 1) * NHALF)
                    for kd in range(4):
                        nc.tensor.matmul(pf[:, 0:NHALF], hn[kd][:, msl],
                                         fcw_sb[:, kd, nsl],
                                         start=(kd == 0), stop=(kd == 3))
                    pad = "outpad" in flags or "obf16" in flags
                    ncol = NHALF if pad else (
                        NHALF if nh == 0 else D - NHALF)
                    osb = outp.tile([128, NHALF], odt, tag="osb")
                    if "rebal" in flags or "rebal3" in flags:
                        nc.scalar.copy(osb[:], pf[:, 0:NHALF])
                    elif "rebal2" in flags and (m + nh) % 2:
                        nc.scalar.copy(osb[:], pf[:, 0:NHALF])
                    else:
                        nc.vector.tensor_copy(osb[:], pf[:, 0:NHALF])
                    if "noout" not in flags:
                        deng = nc.sync
                        if "odmas" in flags:
                            deng = nc.scalar
                        elif "odmago" in flags:
                            deng = nc.gpsimd if (m + nh) % 2 else nc.sync
                        elif "odmag" in flags:
                            deng = nc.gpsimd
                        deng.dma_start(
                            out[ts + m * 128 : ts + (m + 1) * 128,
                                nh * NHALF : nh * NHALF + ncol],
                            osb[:, 0:ncol],
                        )
                return
            pf = fmp.tile([128, 1024], FP32, tag="fm")
            for kd in range(4):
                for nh in range(2):
                    po = pf[:, nh * 512 : nh * 512 + NHALF]
                    nsl = slice(nh * NHALF, (nh + 1) * NHALF)
                    nc.tensor.matmul(po, hn[kd][:, msl], fcw_sb[:, kd, nsl],
                                     start=(kd == 0), stop=(kd == 3))
            osb = outp.tile([128, 2, NHALF], odt, tag="osb")
            pfv = pf[:].rearrange("p (a z) -> p a z", a=2)[:, :, 0:NHALF]
            if "rebal" in flags:
                nc.scalar.copy(osb[:], pfv)
            else:
                nc.vector.tensor_copy(osb[:], pfv)
            if "noout" not in flags:
                deng = nc.sync
                if "odmas" in flags:
                    deng = nc.scalar
                elif "odmag" in flags:
                    deng = nc.gpsimd
                deng.dma_start(
                    out[ts + m * 128 : ts + (m + 1) * 128, 0:D],
                    osb[:].rearrange("p a z -> p (a z)")[:, 0:D],
                )

        # ---- prologue: gates+cell for supertile 0 (no fc yet) ----
        xtiles = x_queue.pop(0)
        h_prev = [emit_k_group(k, 0, xtiles) for k in range(4)]

        # ---- main loop: fc(st) interleaved with gates+cell(st+1) ----
        # unr2: 2 ring periods per For_i iteration halves the per-iteration
        # all-engine-barrier seam (and its HAM re-warm cost)
        unroll = 2 if ("unr2" in flags and reps > 1) else 1
        rep_ctx = (
            tc.For_i(0, reps // unroll, 1, staggered_reset=True)
            if reps > 1
            else None
        )
        if rep_ctx is not None:
            rep_ctx.__enter__()
        for _u in range(unroll):
            for st in range(NST):
                s_next = (st + 1) % NST
                do_gates = (reps > 1) or (st < NST - 1)
                if do_gates:
                    xtiles = x_queue.pop(0)
                    if reps > 1:
                        x_queue.append(emit_x((s_next + 2) % NST))
                    elif s_next + 2 < NST:
                        x_queue.append(emit_x(s_next + 2))
                h_new = []
                for k in range(4):
                    if do_gates:
                        h_new.append(emit_k_group(k, s_next, xtiles))
                    emit_fc(st, h_prev, 2 * k)
                    emit_fc(st, h_prev, 2 * k + 1)
                if do_gates:
                    h_prev = h_new
        if rep_ctx is not None:
            rep_ctx.__exit__(None, None, None)

    nc.compile()
    return nc


_NC_CACHE = []


def get_nc():
    if not _NC_CACHE:
        _NC_CACHE.append(build_nc())
    return _NC_CACHE[0]


def make_in_maps(decoder_inputs, h0, c0, W_ih, W_hh, b_ih, b_hh, fc_w, fc_b):
    di = np.asarray(decoder_inputs, dtype=np.float32)
    h0 = np.asarray(h0, dtype=np.float32)[0]  # [64, 512]
    c0 = np.asarray(c0, dtype=np.float32)[0]
    W_ih = np.asarray(W_ih, dtype=np.float32)
    W_hh = np.asarray(W_hh, dtype=np.float32)
    b_ih = np.asarray(b_ih, dtype=np.float32)
    b_hh = np.asarray(b_hh, dtype=np.float32)
    fc_w = np.asarray(fc_w, dtype=np.float32)

    bc = h0 @ W_hh.T + b_ih + b_hh  # [64, 2048]

    # i/f/o gate weights: single fp8 e4m3 at x64 scale over feats 0..511,
    # SwInterleave DR-pair layout. (g-gate slots are packed too but unused.)
    W_hi8 = (SW * W_ih[:, 0:512]).astype(E4)  # [2048, 512]

    def wpack(w8):
        # SwInterleave layout: arr[k, pair, c*128+j, i] = w8[c*128+(127-j),
        # pair*256 + i*128 + k]  -> [128, 2(pair), 2048(g), 2(ktile)]
        tmp = w8.reshape(16, 128, 2, 2, 128)  # (c, m, pair, i, k)
        return np.ascontiguousarray(
            np.flip(tmp, axis=1).transpose(4, 2, 0, 1, 3).reshape(128, 2, 2048, 2))

    w8_a = wpack(W_hi8)

    # g gate weights bf16: [128(k within ktile), 4(ktile), 512(g rows)]
    wg_a = np.ascontiguousarray(
        W_ih[1024:1536, 0:512].T.astype(BFD).reshape(4, 128, 512).transpose(1, 0, 2))

    # col512 scalars per chunk, pre-scaled by 64 for the fp8 (i/f/o) chunks
    wcol_a = np.empty((128, 16), dtype=np.float32)
    for c in range(16):
        s = SW if (c // 4) != 2 else 1.0
        wcol_a[:, c] = s * W_ih[c * 128 : (c + 1) * 128, 512]
    # row-vector form for the K=1 PE path (bf16)
    wct_a = np.ascontiguousarray(
        wcol_a.T.reshape(1, 2048).astype(BFD))

    # fc weights bf16, [128, 4(k-chunk), 516]
    fc_pad = np.zeros((512, DPAD), dtype=BFD)
    fc_pad[:, 0:D] = fc_w.T.astype(BFD)
    fcw_a = np.ascontiguousarray(
        fc_pad.reshape(4, 128, DPAD).transpose(1, 0, 2))

    in_maps = []
    for core in range(N_CORES):
        bs = core * B_LOC
        xc = di[bs : bs + B_LOC].reshape(TOK, D)
        xt8_a = np.ascontiguousarray(xc.T[0:512].astype(E4))  # [512, TOK]
        xbf_a = np.ascontiguousarray(xc.T[0:512].astype(BFD))  # [512, TOK]
        x5_a = np.ascontiguousarray(xc.T[512:513].astype(np.float32))  # [1, TOK]
        bct = np.ascontiguousarray(
            bc[bs : bs + B_LOC]
            .reshape(B_LOC, 16, 128)
            .transpose(2, 1, 0)
            .reshape(128, -1)
        )
        c0c = np.ascontiguousarray(
            c0[bs : bs + B_LOC]
            .reshape(B_LOC, 4, 128)
            .transpose(2, 1, 0)
            .reshape(128, -1)
        )
        in_maps.append(
            {
                "xt": xt8_a,
                "xb": xbf_a,
                "x5": x5_a,
                "x5h": np.ascontiguousarray(x5_a.astype(BFD)),
                "w8": w8_a,
                "wg": wg_a,
                "wcol": wcol_a,
                "wct": wct_a,
                "fcw": fcw_a,
                "bconst": bct,
                "c0t": c0c,
            }
        )
    return in_maps


def kernel(**inputs):
    in_maps = make_in_maps(**inputs)
    nc = get_nc()
    res = run_bass_kernel_spmd(nc, in_maps, core_ids=list(range(N_CORES)))
    out = np.concatenate(
        [res.results[c]["out"][:, 0:D].astype(np.float32) for c in range(N_CORES)],
        axis=0,
    )
    out = out.reshape(B, T, D)
    out += np.asarray(inputs["fc_b"], dtype=np.float32)  # exact fp32 bias
    return out
